# revision 9
# baseline (speedup 1.0000x reference)
"""CoarseMatching (dual-softmax mutual-NN) Trainium2 kernel, 8-core SPMD.

Math (per batch=1):
  f0n, f1n = L2-normalize(f0_flat), L2-normalize(f1_flat)   # (6400, 256)
  S = f0n @ f1n.T / 0.1                                     # (6400, 6400)
  P = softmax_row(S) * softmax_col(S)
S = cos/0.1 is in [-10, 10], so exp(S) cannot overflow fp32 and no
max-subtraction is needed:
  Z = exp(S); R_i = sum_j Z; C_j = sum_i Z
  P = Z^2 / (R_i * C_j) = (Z * sqrt(1/R_i))^2 * (1/C_j)

Sharding: columns of S/P (the N1 axis) are split 800-per-core across 8
cores; every core holds all 6400 rows.  Row sums R need one small
AllReduce (128x50 fp32); column sums C are core-local.  conf_i =
max_j P_ij comes free from a fused tensor_tensor_reduce (max) in the
final multiply; per-core partial confs are max-combined on the host.

The argmax-dependent outputs (mkpts0/mkpts1/mconf/valid) are only
non-zero when conf > 0.2 somewhere; the host epilogue short-circuits to
zeros when max(conf) is far below the threshold (exact in that case)
and otherwise falls back to a full numpy epilogue on the assembled P.
"""

import numpy as np

import concourse.bass as bass
import concourse.bacc as bacc
import concourse.mybir as mybir
import concourse.tile as tile
from concourse.bass_utils import run_bass_kernel_spmd

F32 = mybir.dt.float32
AF = mybir.ActivationFunctionType
ALU = mybir.AluOpType

N_CORES = 8
C = 256                  # channels
N = 6400                 # N0 == N1 (80*80)
COLS = N // N_CORES      # 800 columns per core
RT = N // 128            # 50 row tiles of 128
W0 = 80
TEMP = 0.1
THRESHOLD = 0.2
CHUNK = 512              # f0 column chunk (i-axis) for streamed normalize
NCHUNK = (N + CHUNK - 1) // CHUNK          # 13 (12x512 + 1x256)
NSPLIT = ((0, 512), (512, 288))            # 800-col split on psum banks


def _emit_body(tc, f0_d, f1_d, p_d, conf_d, pools, collective=True):
    nc = tc.nc
    (persist, zpool, f0pool, sqpool, rspool, ypool, ppool,
     spsum, cspsum, nqpsum, drampool) = pools

    ones = persist.tile([128, 128], F32, name="ones", tag="ones")
    nc.vector.memset(ones[:], 1.0)

    # ---- f1 prep: load, compute 1/||f1_j|| as broadcast, normalize ----
    f1t = [persist.tile([128, COLS], F32, name=f"f1_{kt}", tag=f"f1_{kt}") for kt in range(2)]
    for kt in range(2):
        nc.sync.dma_start(f1t[kt][:], f1_d[kt * 128:(kt + 1) * 128, :])
    f1sq = [sqpool.tile([128, COLS], F32, name="sq", tag="sq") for _ in range(2)]
    for kt in range(2):
        nc.vector.tensor_mul(f1sq[kt][:], f1t[kt][:], f1t[kt][:])
    rs1B = persist.tile([128, COLS], F32, name="rs1B", tag="rs1B")
    for off, nn in NSPLIT:
        nq = nqpsum.tile([128, CHUNK], F32, name="nq", tag="nq")
        for kt in range(2):
            nc.tensor.matmul(nq[:, 0:nn], ones[:], f1sq[kt][:, off:off + nn],
                             start=(kt == 0), stop=(kt == 1))
        lnn = rspool.tile([128, CHUNK], F32, name="lnn", tag="ln")
        nc.scalar.activation(lnn[:, 0:nn], nq[:, 0:nn], AF.Ln)
        nc.scalar.activation(rs1B[:, off:off + nn], lnn[:, 0:nn], AF.Exp,
                             scale=-0.5)
    for kt in range(2):
        nc.vector.tensor_mul(f1t[kt][:], f1t[kt][:], rs1B[:])

    # ---- main loop: stream f0 chunks, normalize, matmul, exp ----
    rpart = persist.tile([128, RT], F32, name="rpart", tag="rpart")
    cs = cspsum.tile([128, COLS], F32, name="cs", tag="cs")
    z_tiles = []
    f0n_cur = None
    for r in range(RT):
        c = r // 4
        if r % 4 == 0:
            # prep f0 chunk c: columns [512c, 512c+w)
            c0 = c * CHUNK
            w = min(CHUNK, N - c0)
            f0n_cur = [f0pool.tile([128, CHUNK], F32, name="f0c", tag="f0") for _ in range(2)]
            for kt in range(2):
                nc.sync.dma_start(f0n_cur[kt][:, 0:w],
                                  f0_d[kt * 128:(kt + 1) * 128, c0:c0 + w])
            sq = [sqpool.tile([128, CHUNK], F32, name="sq", tag="sq") for _ in range(2)]
            for kt in range(2):
                nc.vector.tensor_mul(sq[kt][:, 0:w], f0n_cur[kt][:, 0:w],
                                     f0n_cur[kt][:, 0:w])
            nq = nqpsum.tile([128, CHUNK], F32, name="nq", tag="nq")
            for kt in range(2):
                nc.tensor.matmul(nq[:, 0:w], ones[:], sq[kt][:, 0:w],
                                 start=(kt == 0), stop=(kt == 1))
            lnn = rspool.tile([128, CHUNK], F32, name="lnn", tag="ln")
            nc.scalar.activation(lnn[:, 0:w], nq[:, 0:w], AF.Ln)
            rs0 = rspool.tile([128, CHUNK], F32, name="rs0", tag="rs")
            nc.scalar.activation(rs0[:, 0:w], lnn[:, 0:w], AF.Exp, scale=-0.5)
            for kt in range(2):
                nc.vector.tensor_mul(f0n_cur[kt][:, 0:w], f0n_cur[kt][:, 0:w],
                                     rs0[:, 0:w])
        roff = (r % 4) * 128
        s_ps = spsum.tile([128, COLS], F32, name="s_ps", tag="s")
        for kt in range(2):
            for off, nn in NSPLIT:
                nc.tensor.matmul(s_ps[:, off:off + nn],
                                 f0n_cur[kt][:, roff:roff + 128],
                                 f1t[kt][:, off:off + nn],
                                 start=(kt == 0), stop=(kt == 1))
        z = zpool.tile([128, COLS], F32, name="z", tag="z")
        nc.scalar.activation(z[:], s_ps[:], AF.Exp, scale=1.0 / TEMP,
                             accum_out=rpart[:, r:r + 1])
        z_tiles.append(z)
        for off, nn in NSPLIT:
            nc.tensor.matmul(cs[:, off:off + nn], ones[:], z[:, off:off + nn],
                             start=(r == 0), stop=(r == RT - 1),
                             skip_group_check=True)

    # ---- AllReduce row sums over the 8 cores ----
    rin = drampool.tile([128, RT], F32, name="rin", tag="rin")
    rout = drampool.tile([128, RT], F32, name="rout", tag="rout")
    nc.gpsimd.dma_start(rin[:], rpart[:])
    if collective:
        nc.gpsimd.collective_compute(
            "AllReduce", ALU.add,
            replica_groups=[list(range(N_CORES))],
            ins=[rin.opt()], outs=[rout.opt()],
        )
    else:
        nc.gpsimd.dma_start(rout[:], rin[:])
    rtot = persist.tile([128, RT], F32, name="rtot", tag="rtot")
    nc.gpsimd.dma_start(rtot[:], rout[:])

    # sqrt(1/R) and 1/R per-partition
    lnr = persist.tile([128, RT], F32, name="lnr", tag="lnr")
    nc.scalar.activation(lnr[:], rtot[:], AF.Ln)
    squ = persist.tile([128, RT], F32, name="squ", tag="squ")
    nc.scalar.activation(squ[:], lnr[:], AF.Exp, scale=-0.5)
    usc = persist.tile([128, RT], F32, name="usc", tag="usc")
    nc.scalar.activation(usc[:], lnr[:], AF.Exp, scale=-1.0)
    # vb = 1/sqrt(C) broadcast (applied before the square, so sqrt)
    vb = persist.tile([128, COLS], F32, name="vb", tag="vb")
    nc.scalar.activation(vb[:], cs[:], AF.Ln)
    nc.scalar.activation(vb[:], vb[:], AF.Exp, scale=-0.5)

    # ---- phase 2: T = Z/sqrt(C); P = (T*sqrt(1/R))^2, row-sums fused ----
    # conf_sb accumulates sum_j P per row (an upper bound on max_j P,
    # summed across cores on the host for the threshold decision).
    # Square+scale runs on ACT for 5 of 6 tiles, DVE for the rest, to
    # balance engine spans.
    conf_sb = persist.tile([128, RT], F32, name="conf_sb", tag="conf")
    for r in range(RT):
        t = ypool.tile([128, COLS], F32, name="t", tag="y")
        nc.vector.tensor_mul(t[:], z_tiles[r][:], vb[:])
        pt = ppool.tile([128, COLS], F32, name="pt", tag="p")
        if r % 6 != 5:
            nc.scalar.activation(pt[:], t[:], AF.Square,
                                 scale=squ[:, r:r + 1],
                                 accum_out=conf_sb[:, r:r + 1])
        else:
            nc.vector.tensor_mul(pt[:], t[:], t[:])
            nc.vector.tensor_scalar(
                out=pt[:], in0=pt[:], scalar1=usc[:, r:r + 1], scalar2=0.0,
                op0=ALU.mult, op1=ALU.add, accum_out=conf_sb[:, r:r + 1])
        nc.sync.dma_start(p_d[r * 128:(r + 1) * 128, :], pt[:])
    nc.sync.dma_start(conf_d[:, :], conf_sb[:])


def build(repeat: int = 1, collective: bool = True):
    nc = bacc.Bacc("TRN2", target_bir_lowering=False, debug=False,
                   num_devices=N_CORES)
    f0_d = nc.dram_tensor("f0_full", (C, N), F32, kind="ExternalInput")
    f1_d = nc.dram_tensor("f1_part", (C, COLS), F32, kind="ExternalInput")
    p_d = nc.dram_tensor("p_part", (N, COLS), F32, kind="ExternalOutput")
    conf_d = nc.dram_tensor("conf_part", (128, RT), F32, kind="ExternalOutput")
    with tile.TileContext(nc) as tc:
        with (
            tc.tile_pool(name="persist", bufs=1) as persist,
            tc.tile_pool(name="zslab", bufs=RT) as zpool,
            tc.tile_pool(name="f0", bufs=4) as f0pool,
            tc.tile_pool(name="sq", bufs=2) as sqpool,
            tc.tile_pool(name="rs", bufs=2) as rspool,
            tc.tile_pool(name="y", bufs=2) as ypool,
            tc.tile_pool(name="p", bufs=2) as ppool,
            tc.tile_pool(name="spsum", bufs=2, space="PSUM") as spsum,
            tc.tile_pool(name="cspsum", bufs=1, space="PSUM") as cspsum,
            tc.tile_pool(name="nqpsum", bufs=2, space="PSUM") as nqpsum,
            tc.tile_pool(name="dram", bufs=2, space="DRAM") as drampool,
        ):
            pools = (persist, zpool, f0pool, sqpool, rspool, ypool, ppool,
                     spsum, cspsum, nqpsum, drampool)
            for _ in range(repeat):
                _emit_body(tc, f0_d, f1_d, p_d, conf_d, pools, collective)
    nc.compile()
    return nc


_CACHED = {}


def _get_nc(repeat: int = 1):
    if repeat not in _CACHED:
        _CACHED[repeat] = build(repeat)
    return _CACHED[repeat]


def _epilogue(P, conf):
    ar0 = np.arange(N)
    if conf.max() > THRESHOLD * 0.95:
        # general path (never taken for inputs whose conf is far below
        # the threshold): recompute the epilogue exactly from P
        P0 = P[0]
        conf = P0.max(-1)
        idx0 = P0.argmax(-1)
        idx1 = P0.argmax(0)
        mutual = idx1[idx0] == ar0
        valid = mutual & (conf > THRESHOLD)
        vf = valid[:, None].astype(np.float32)
        mk0 = np.stack([ar0 % W0, ar0 // W0], 1).astype(np.float32) * vf
        mk1 = np.stack([idx0 % W0, idx0 // W0], 1).astype(np.float32) * vf
        mconf = np.where(valid, conf, 0.0).astype(np.float32)
        return mk0, mk1, mconf, valid
    valid = np.zeros(N, bool)
    zeros2 = np.zeros((N, 2), np.float32)
    return zeros2, zeros2.copy(), np.zeros(N, np.float32), valid


def kernel(f0, f1):
    f0 = np.ascontiguousarray(np.asarray(f0, np.float32).reshape(C, N))
    f1 = np.ascontiguousarray(np.asarray(f1, np.float32).reshape(C, N))
    nc = _get_nc(1)
    in_maps = [
        {"f0_full": f0,
         "f1_part": np.ascontiguousarray(f1[:, k * COLS:(k + 1) * COLS])}
        for k in range(N_CORES)
    ]
    res = run_bass_kernel_spmd(nc, in_maps, core_ids=list(range(N_CORES)))
    parts = res.results
    P = np.concatenate([parts[k]["p_part"] for k in range(N_CORES)],
                       axis=1)[None]
    confs = np.stack([parts[k]["conf_part"] for k in range(N_CORES)])
    # row-sums of P summed over cores: an upper bound on conf_i = max_j P
    conf = confs.sum(axis=0).T.reshape(N)     # bound[i], i = 128*r + p
    mk0, mk1, mconf, valid = _epilogue(P, conf)
    return (P, mk0, mk1, mconf, valid)
